# revision 1
# baseline (speedup 1.0000x reference)
"""Distributed Trainium2 kernel for a GATv2 layer + BN + global-mean-pool + classifier.

Math (reference, heads=1):
    xl = x@Wl + bl ; xr = x@Wr + br
    logit_e = att . leaky_relu(xl[src_e] + xr[dst_e], 0.2)
    a_e     = segment_softmax(logit_e over dst)
    out_i   = sum_{e: dst=i} a_e * xl[src_e] ; out = out + bias1
    h       = BN(out) ; g = mean_i h ; y = softmax(g@Wc + bc)

The output is a global mean over nodes and BN is affine per feature, so
per-node outputs never materialize:
    y = softmax( ((S/N)*A + B) @ Wc + bc ),  S = sum_e a_e * xl[src_e],
    A = gamma/sqrt(var+eps), B = (bias1 - mu)*A + beta.

Attention weights v = att are folded into the gather tables host-side:
    v_f * lrelu(z_f) = sign_f * lrelu(|v_f| z_f)
with features permuted so positive-sign features occupy columns [0,PP).
The gathered table is xg = perm(|v| (.) xl); since |v|>0 this is inverted
through the head constants (A' = A[perm]/(N |v|), Wc' = Wc[perm]), so the
same gathered rows serve both the logits and the weighted sum.

Distribution over 8 cores: nodes sharded contiguously; edges sharded by dst
and packed (whole per-dst segments, one partition each) into a
[128 partitions x L slots] grid. Per-edge logits come from bulk dma_gather
(int16 indices -> two half-tables, invalid side pointing at a -1e30 dummy
row, merged with one elementwise max). Segment softmax = forward masked
scan + reversed masked max-scan. A second gather pass computes
partial = sum_slots w * xg[src] in PSUM; AllReduce + a tiny head finish.
"""

import math
import os

import ml_dtypes
import numpy as np

import concourse.bass as bass
import concourse.bacc as bacc
import concourse.mybir as mybir
import concourse.tile as tile

M = 8  # cores
F = 128
NCLS = 5
BN_EPS = 1e-5

BF16 = ml_dtypes.bfloat16


def _wrap_idx(seq):
    """[N] int array -> [128, N//16] int16 wrap layout (16-partition groups,
    replicated across the 8 gpsimd cores)."""
    n = seq.shape[0]
    assert n % 16 == 0
    w = seq.reshape(n // 16, 16).T.astype(np.int16)
    return np.tile(w, (8, 1))


def _segment_fields(sorted_key):
    n = sorted_key.shape[0]
    start = np.ones(n, bool)
    start[1:] = sorted_key[1:] != sorted_key[:-1]
    end = np.ones(n, bool)
    end[:-1] = sorted_key[1:] != sorted_key[:-1]
    idx = np.arange(n, dtype=np.int64)
    first = np.where(start, idx, 0)
    first = np.maximum.accumulate(first)
    return start, end, idx - first


def prep_host(x, edge_index, Wl, bl, Wr, br, att, bias1,
              bn_gamma, bn_beta, bn_mean, bn_var, Wc, bc):
    N = x.shape[0]
    npc = N // M
    assert npc * M == N
    NPC = ((npc + 1 + 127) // 128) * 128  # always >= 1 pad row (dummy)
    CH = NPC // 128
    NG = M * NPC
    HALF = NG // 2
    DUM = npc  # first pad row of each core's shard holds -1e30

    src = np.concatenate([edge_index[0], np.arange(N, dtype=np.int64)])
    dst = np.concatenate([edge_index[1], np.arange(N, dtype=np.int64)])

    # ---- attention folding ----
    v = np.asarray(att[0], np.float64)
    posm = v >= 0
    perm = np.argsort(~posm, kind="stable")
    PP = int(posm.sum())
    assert 0 < PP < F, f"degenerate attention sign split PP={PP}"
    absv = np.abs(v[perm])
    Wg_l = (Wl[:, perm] * absv[None, :]).astype(np.float32)
    bg_l = (bl[perm] * absv).astype(np.float32)
    Wg_r = (Wr[:, perm] * absv[None, :]).astype(np.float32)
    bg_r = (br[perm] * absv).astype(np.float32)

    nodes = np.arange(N, dtype=np.int64)
    cN = nodes // npc
    lN = nodes % npc

    # ---- dst-grid: greedy LPT node->partition packing ----
    deg = np.bincount(dst, minlength=N)
    pnode = np.zeros(N, np.int64)    # partition of each node
    soff = np.zeros(N, np.int64)     # slot offset of node's segment
    Lmax = 0
    for k in range(M):
        dk = deg[k * npc:(k + 1) * npc]
        order_n = np.argsort(-dk, kind="stable")
        loads = np.zeros(128, np.int64)
        for g in order_n:
            p = int(np.argmin(loads))
            pnode[k * npc + g] = p
            soff[k * npc + g] = loads[p]
            loads[p] += dk[g]
        Lmax = max(Lmax, int(loads.max()))
    # L must be a multiple of 128 so chunk idx sequences tile evenly
    L = ((max(Lmax, 128) + 127) // 128) * 128

    order = np.argsort(dst, kind="stable")
    ds = dst[order]
    ss = src[order]
    d_start, d_end, q = _segment_fields(ds)
    cd = ds // npc
    dls = ds - cd * npc
    pd = pnode[ds]
    t = soff[ds] + q
    assert t.max() < L

    srcrow = (ss // npc) * NPC + (ss % npc)  # global padded row
    iP = np.zeros((M, 128, L), np.int64)              # pair row (2 nodes/row)
    par = np.zeros((M, 128, L), np.float32)           # which half of the pair
    iP[cd, pd, t] = srcrow >> 1
    par[cd, pd, t] = (srcrow & 1).astype(np.float32)

    iX = np.zeros((M, 128, L), np.int64)              # xr local row
    iX[cd, pd, t] = dls
    mask_f = np.zeros((M, 128, L), np.float32)
    mask_r = np.zeros((M, 128, L), np.float32)
    mask_v = np.zeros((M, 128, L), np.float32)
    mask_f[cd, pd, t] = (~d_start).astype(np.float32)
    mask_r[cd, pd, t] = (~d_end).astype(np.float32)
    mask_v[cd, pd, t] = 1.0

    # gather index order: position i -> slot (p = i%128, t = i//128)
    def to_wrap(a):  # [128, L] -> wrap over i-sequence
        seq = a.T.reshape(-1)  # i = t*128 + p
        return _wrap_idx(seq)

    iP_w = np.stack([to_wrap(iP[k]) for k in range(M)])
    iX_w = np.stack([to_wrap(iX[k]) for k in range(M)])

    # ---- head constants (de-permuted / de-scaled) ----
    A = bn_gamma.astype(np.float64) / np.sqrt(bn_var.astype(np.float64) + BN_EPS)
    Ap = (A[perm] / (N * absv)).astype(np.float32).reshape(F, 1)
    Bp = ((bias1 - bn_mean).astype(np.float64) * A + bn_beta)[perm] \
        .astype(np.float32).reshape(F, 1)
    Wcp = Wc[perm, :].astype(np.float32)

    # ---- per-core x^T (padded, bf16) ----
    xT = np.zeros((M, 128, NPC), BF16)
    for k in range(M):
        xT[k, :, :npc] = x[k * npc:(k + 1) * npc].T.astype(BF16)

    meta = dict(NPC=NPC, CH=CH, NG=NG, L=L, PP=PP, N=N, DUM=DUM)

    in_maps = []
    for k in range(M):
        in_maps.append({
            "xT": np.ascontiguousarray(xT[k]),
            "Wgl": Wg_l.astype(BF16),
            "bgl": bg_l.reshape(1, F).astype(BF16),
            "Wgr": Wg_r.astype(BF16),
            "bgr": bg_r.reshape(1, F).astype(BF16),
            "iP": np.ascontiguousarray(iP_w[k]),
            "iX": np.ascontiguousarray(iX_w[k]),
            "par": np.ascontiguousarray(par[k]),
            "mask_f": np.ascontiguousarray(mask_f[k]),
            "mask_r": np.ascontiguousarray(mask_r[k]),
            "mask_v": np.ascontiguousarray(mask_v[k]),
            "Ap": Ap,
            "Bp": Bp,
            "Wcp": Wcp,
            "bc": bc.reshape(1, NCLS).astype(np.float32),
        })
    return in_maps, meta


def build(meta, nchunks=24):
    stage = int(os.environ.get("KERNEL_STAGE", "3"))
    NPC, CH, NG, L, PP, DUM = (meta[k] for k in ("NPC", "CH", "NG", "L", "PP", "DUM"))
    HALF = NG // 2
    while nchunks > 1 and L % (nchunks * 8):
        nchunks -= 1
    KC = L // nchunks      # slot-columns per chunk
    NI = KC * 128          # gather indices per call
    LW = (L * 128) // 16   # wrap-index array width

    dt = mybir.dt
    alu = mybir.AluOpType
    act = mybir.ActivationFunctionType
    rg = [list(range(M))]

    nc = bacc.Bacc("TRN2", target_bir_lowering=False, debug=False, num_devices=M)

    def p_in(name, shape, d):
        return nc.dram_tensor(name, shape, d, kind="ExternalInput").ap()

    xT = p_in("xT", [128, NPC], dt.bfloat16)
    Wgl = p_in("Wgl", [F, F], dt.bfloat16)
    bgl = p_in("bgl", [1, F], dt.bfloat16)
    Wgr = p_in("Wgr", [F, F], dt.bfloat16)
    bgr = p_in("bgr", [1, F], dt.bfloat16)
    iP = p_in("iP", [128, LW], dt.int16)
    iX = p_in("iX", [128, LW], dt.int16)
    par = p_in("par", [128, L], dt.float32)
    mask_f = p_in("mask_f", [128, L], dt.float32)
    mask_r = p_in("mask_r", [128, L], dt.float32)
    mask_v = p_in("mask_v", [128, L], dt.float32)
    Ap = p_in("Ap", [F, 1], dt.float32)
    Bp = p_in("Bp", [F, 1], dt.float32)
    Wcp = p_in("Wcp", [F, NCLS], dt.float32)
    bc = p_in("bc", [1, NCLS], dt.float32)
    out = nc.dram_tensor("out", [1, NCLS], dt.float32, kind="ExternalOutput").ap()

    with tile.TileContext(nc) as tc:
        with (
            tc.tile_pool(name="dram", bufs=1, space="DRAM") as dpool,
            tc.tile_pool(name="sbp", bufs=1) as sbp,
            tc.tile_pool(name="sbw", bufs=2) as sbw,
            tc.tile_pool(name="ps2", bufs=2, space="PSUM") as pp,
            tc.tile_pool(name="ps1", bufs=1, space="PSUM") as pp1,
        ):
            xg_loc = dpool.tile([NPC, F], dt.bfloat16)
            xr_loc = dpool.tile([NPC, 2 * F], dt.bfloat16)
            xg_full = dpool.tile([NG, F], dt.bfloat16, addr_space="Shared")
            xls_scr = dpool.tile([128, L * 2 * F], dt.bfloat16)
            part_loc = dpool.tile([1, F], dt.float32)
            pooled = dpool.tile([1, F], dt.float32, addr_space="Shared")

            # ---- persistent SBUF ----
            xT_sb = sbp.tile([128, NPC], dt.bfloat16)
            nc.sync.dma_start(xT_sb[:], xT)
            wt = {}
            for nm, apin, sh in (("Wgl", Wgl, [F, F]), ("bgl", bgl, [1, F]),
                                 ("Wgr", Wgr, [F, F]), ("bgr", bgr, [1, F])):
                tl = sbp.tile(sh, dt.bfloat16, tag=nm)
                nc.sync.dma_start(tl[:], apin)
                wt[nm] = tl
            ones_sb = sbp.tile([1, F], dt.bfloat16)
            nc.vector.memset(ones_sb[:], 1.0)

            iP_sb = sbp.tile([128, LW], dt.int16)
            nc.sync.dma_start(iP_sb[:], iP)
            iX_sb = sbp.tile([128, LW], dt.int16)
            nc.sync.dma_start(iX_sb[:], iX)
            par_sb = sbp.tile([128, L], dt.float32)
            nc.sync.dma_start(par_sb[:], par)
            mf_sb = sbp.tile([128, L], dt.float32)
            nc.sync.dma_start(mf_sb[:], mask_f)
            mr_sb = sbp.tile([128, L], dt.float32)
            nc.sync.dma_start(mr_sb[:], mask_r)
            mv_sb = sbp.tile([128, L], dt.float32)
            nc.sync.dma_start(mv_sb[:], mask_v)

            logits_sb = sbp.tile([128, L], dt.float32)
            if stage >= 2:
                E_sb = sbp.tile([128, L], dt.float32)
                S_sb = sbp.tile([128, L], dt.float32)
                D_sb = sbp.tile([128, L], dt.float32)
                w_sb = sbp.tile([128, L], dt.float32)
                wb_sb = sbp.tile([128, L], dt.bfloat16)
                wb1_sb = sbp.tile([128, L], dt.bfloat16)

            # ---- stage A: node tables ----
            for ci in range(CH):
                lhs = xT_sb[:, 128 * ci:128 * (ci + 1)]
                for wn, bn_ in (("Wgl", "bgl"), ("Wgr", "bgr")):
                    ps = pp.tile([128, F], dt.float32, tag="psA")
                    nc.tensor.matmul(ps[:], lhsT=lhs, rhs=wt[wn][:],
                                     start=True, stop=False)
                    nc.tensor.matmul(ps[:], lhsT=ones_sb[:], rhs=wt[bn_][:],
                                     start=False, stop=True)
                    ob = sbw.tile([128, F], dt.bfloat16, tag="stA")
                    nc.vector.tensor_copy(ob[:], ps[:])
                    rows = slice(128 * ci, 128 * (ci + 1))
                    if wn == "Wgl":
                        nc.sync.dma_start(xg_loc[rows, :], ob[:])
                    else:
                        nc.sync.dma_start(xr_loc[rows, 0:F], ob[:])
                        nc.sync.dma_start(xr_loc[rows, F:2 * F], ob[:])
            nc.gpsimd.collective_compute(
                "AllGather", alu.bypass, replica_groups=rg,
                ins=[xg_loc.opt()], outs=[xg_full.opt()])

            tab_pair = xg_full[:].rearrange("(a two) f -> a (two f)", two=2)

            def gather(dst_tile, tab, idx_sb, c):
                nc.gpsimd.dma_gather(
                    out_ap=dst_tile[:].rearrange("p (b f) -> p b f", f=2 * F),
                    in_ap=tab,
                    idxs_ap=idx_sb[:, (NI // 16) * c:(NI // 16) * (c + 1)],
                    num_idxs=NI, num_idxs_reg=NI, elem_size=2 * F,
                    single_packet=False)

            # ---- pass 1: logits ----
            W2 = 2 * F * KC
            for c in range(nchunks):
                sl = slice(c * KC, (c + 1) * KC)
                gp = sbw.tile([128, W2], dt.bfloat16, tag="gp", bufs=3)
                gx = sbw.tile([128, W2], dt.bfloat16, tag="gx", bufs=3)
                gather(gp, tab_pair, iP_sb, c)
                gather(gx, xr_loc[:], iX_sb, c)
                # stash the raw gathered pairs for pass 2
                nc.sync.dma_start(xls_scr[:, W2 * c:W2 * (c + 1)], gp[:])
                nc.vector.tensor_tensor(out=gx[:], in0=gx[:], in1=gp[:],
                                        op=alu.add)
                zab = sbw.tile([128, W2], dt.bfloat16, tag="zab", bufs=3)
                nc.scalar.activation(zab[:], gx[:], act.Abs, scale=0.4)
                nc.vector.scalar_tensor_tensor(
                    out=zab[:], in0=gx[:], scalar=0.6, in1=zab[:],
                    op0=alu.mult, op1=alu.add)
                m3 = zab[:].rearrange("p (k f) -> p k f", f=2 * F)
                lgp = sbw.tile([128, KC], dt.float32, tag="lgp")
                lgn = sbw.tile([128, KC], dt.float32, tag="lgn")
                lgp1 = sbw.tile([128, KC], dt.float32, tag="lgp1")
                lgn1 = sbw.tile([128, KC], dt.float32, tag="lgn1")
                nc.vector.tensor_reduce(lgp[:], m3[:, :, 0:PP],
                                        axis=mybir.AxisListType.X, op=alu.add)
                nc.vector.tensor_reduce(lgn[:], m3[:, :, PP:F],
                                        axis=mybir.AxisListType.X, op=alu.add)
                nc.vector.tensor_reduce(lgp1[:], m3[:, :, F:F + PP],
                                        axis=mybir.AxisListType.X, op=alu.add)
                nc.vector.tensor_reduce(lgn1[:], m3[:, :, F + PP:2 * F],
                                        axis=mybir.AxisListType.X, op=alu.add)
                # a = p0-n0 ; b = p1-n1 ; logit = a + par*(b-a)
                nc.vector.tensor_tensor(out=lgp[:], in0=lgp[:], in1=lgn[:],
                                        op=alu.subtract)
                nc.vector.tensor_tensor(out=lgp1[:], in0=lgp1[:], in1=lgn1[:],
                                        op=alu.subtract)
                nc.vector.tensor_tensor(out=lgp1[:], in0=lgp1[:], in1=lgp[:],
                                        op=alu.subtract)
                nc.vector.tensor_tensor(out=lgp1[:], in0=lgp1[:],
                                        in1=par_sb[:, sl], op=alu.mult)
                nc.vector.tensor_tensor(out=logits_sb[:, sl], in0=lgp[:],
                                        in1=lgp1[:], op=alu.add)

            # ---- segment softmax ----
            if stage < 2:
                osb0 = sbp.tile([1, NCLS], dt.float32)
                nc.vector.tensor_reduce(osb0[:], logits_sb[0:1, 0:NCLS * 20].rearrange("o (a b) -> o a b", a=NCLS), axis=mybir.AxisListType.X, op=alu.add)
                nc.sync.dma_start(out, osb0[:])
            if stage >= 2:
                nc.scalar.activation(E_sb[:], logits_sb[:], act.Exp)
                nc.vector.tensor_tensor_scan(
                    out=S_sb[:], data0=mf_sb[:], data1=E_sb[:], initial=0.0,
                    op0=alu.mult, op1=alu.add)
                nc.vector.tensor_tensor_scan(
                    out=D_sb[:, ::-1], data0=mr_sb[:, ::-1], data1=S_sb[:, ::-1],
                    initial=0.0, op0=alu.mult, op1=alu.max)
                nc.vector.reciprocal(D_sb[:], D_sb[:])
                nc.vector.tensor_tensor(out=w_sb[:], in0=E_sb[:], in1=D_sb[:],
                                        op=alu.mult)
                nc.vector.tensor_tensor(out=w_sb[:], in0=w_sb[:], in1=mv_sb[:],
                                        op=alu.mult)
                # w1 = w*par ; w0 = w - w1
                nc.vector.tensor_tensor(out=S_sb[:], in0=w_sb[:], in1=par_sb[:],
                                        op=alu.mult)
                nc.vector.tensor_tensor(out=D_sb[:], in0=w_sb[:], in1=S_sb[:],
                                        op=alu.subtract)
                nc.vector.tensor_copy(wb_sb[:], D_sb[:])      # w0 bf16
                nc.vector.tensor_copy(wb1_sb[:], S_sb[:])     # w1 bf16

            # ---- pass 2: partial = sum_slots w * xg[src] ----
            if stage == 2:
                osb1 = sbp.tile([1, NCLS], dt.float32)
                nc.vector.tensor_reduce(osb1[:], w_sb[0:1, 0:NCLS * 20].rearrange("o (a b) -> o a b", a=NCLS), axis=mybir.AxisListType.X, op=alu.add)
                nc.sync.dma_start(out, osb1[:])
            if stage >= 3:
                pacc = pp1.tile([F, 1], dt.float32, tag="pacc")
                first = True
                for c in range(nchunks):
                    gp = sbw.tile([128, W2], dt.bfloat16, tag="gp", bufs=3)
                    nc.sync.dma_start(gp[:], xls_scr[:, W2 * c:W2 * (c + 1)])
                    for b in range(KC):
                        col = slice(c * KC + b, c * KC + b + 1)
                        nc.tensor.matmul(
                            pacc[:], lhsT=gp[:, 2 * F * b:2 * F * b + F],
                            rhs=wb_sb[:, col], start=first, stop=False)
                        first = False
                        nc.tensor.matmul(
                            pacc[:], lhsT=gp[:, 2 * F * b + F:2 * F * (b + 1)],
                            rhs=wb1_sb[:, col], start=False,
                            stop=(c == nchunks - 1 and b == KC - 1))

                part_sb = sbp.tile([F, 1], dt.float32)
                nc.vector.tensor_copy(part_sb[:], pacc[:])
                # store the [F] partial as a flat row in DRAM
                nc.sync.dma_start(part_loc[:].rearrange("o f -> f o"), part_sb[:])

                nc.gpsimd.collective_compute(
                    "AllReduce", alu.add, replica_groups=rg,
                    ins=[part_loc.opt()], outs=[pooled.opt()])

                # ---- head ----
                pool_sb = sbp.tile([F, 1], dt.float32)
                nc.sync.dma_start(pool_sb[:], pooled[:].rearrange("o f -> f o"))
                Ap_sb = sbp.tile([F, 1], dt.float32)
                nc.sync.dma_start(Ap_sb[:], Ap)
                Bp_sb = sbp.tile([F, 1], dt.float32)
                nc.sync.dma_start(Bp_sb[:], Bp)
                Wc_sb = sbp.tile([F, NCLS], dt.float32)
                nc.sync.dma_start(Wc_sb[:], Wcp)
                bc_sb = sbp.tile([1, NCLS], dt.float32)
                nc.sync.dma_start(bc_sb[:], bc)
                h_sb = sbp.tile([F, 1], dt.float32)
                nc.vector.scalar_tensor_tensor(
                    out=h_sb[:], in0=pool_sb[:], scalar=Ap_sb[:, 0:1], in1=Bp_sb[:],
                    op0=alu.mult, op1=alu.add)
                one1 = sbp.tile([1, 1], dt.float32)
                nc.vector.memset(one1[:], 1.0)
                hp = pp1.tile([1, NCLS], dt.float32, tag="hp")
                nc.tensor.matmul(hp[:], lhsT=h_sb[:], rhs=Wc_sb[:], start=True,
                                 stop=False)
                nc.tensor.matmul(hp[:], lhsT=one1[:], rhs=bc_sb[:], start=False,
                                 stop=True)
                eh = sbp.tile([1, NCLS], dt.float32)
                nc.scalar.activation(eh[:], hp[:], act.Exp)
                den = sbp.tile([1, 1], dt.float32)
                nc.vector.tensor_reduce(den[:], eh[:], axis=mybir.AxisListType.X,
                                        op=alu.add)
                rden = sbp.tile([1, 1], dt.float32)
                nc.vector.reciprocal(rden[:], den[:])
                osb = sbp.tile([1, NCLS], dt.float32)
                nc.vector.tensor_scalar(out=osb[:], in0=eh[:], scalar1=rden[:, 0:1],
                                        scalar2=None, op0=alu.mult)
                nc.sync.dma_start(out, osb[:])

    nc.compile()
    return nc


# --------------------------------------------------------------------------
# public entry point
# --------------------------------------------------------------------------

_CACHE = {}


def _install_ntff_hook():
    """Provide antenv.axon_hooks + the ctypes NTFF hook when the image lacks
    them, so run_bass_kernel_spmd(trace=True) can capture exec_time_ns."""
    import contextlib
    import ctypes
    import sys
    import types

    try:
        import antenv.axon_hooks  # noqa: F401
        return
    except ImportError:
        pass
    try:
        import antenv
    except ImportError:
        return
    holder = [None]
    mod = types.ModuleType("antenv.axon_hooks")
    mod.set_axon_ntff_profile_hook = lambda h: holder.__setitem__(0, h)
    mod.get_axon_ntff_profile_hook = lambda: holder[0]
    sys.modules["antenv.axon_hooks"] = mod
    antenv.axon_hooks = mod

    so_path = "/opt/axon/libaxon_pjrt.so"
    if os.path.exists(so_path):
        lib = ctypes.CDLL(so_path)
        if hasattr(lib, "axon_start_nrt_profile"):
            lib.axon_start_nrt_profile.argtypes = [
                ctypes.POINTER(ctypes.c_int64), ctypes.c_size_t]
            lib.axon_start_nrt_profile.restype = ctypes.c_int64
            lib.axon_stop_nrt_profile.argtypes = [ctypes.c_char_p]
            lib.axon_stop_nrt_profile.restype = ctypes.c_int64

            @contextlib.contextmanager
            def _hook(output_dir, device_ids):
                import jax
                jax.devices()
                if device_ids:
                    ids = (ctypes.c_int64 * len(device_ids))(*device_ids)
                    rc = lib.axon_start_nrt_profile(ids, len(device_ids))
                else:
                    rc = lib.axon_start_nrt_profile(None, 0)
                if rc != 0:
                    raise RuntimeError(f"axon_start_nrt_profile rc={rc}")
                try:
                    yield
                finally:
                    n = lib.axon_stop_nrt_profile(str(output_dir).encode())
                    print(f"ntff profile: {n} file(s) -> {output_dir}")

            mod.set_axon_ntff_profile_hook(_hook)

    import concourse.bass_utils as bu
    bu.upload_artifacts = lambda tmpdir: "local://" + str(tmpdir)


def kernel(**inputs):
    from concourse.bass_utils import run_bass_kernel_spmd

    if bool(int(os.environ.get("KERNEL_TRACE", "0"))):
        _install_ntff_hook()
    inputs = {k: np.asarray(v) for k, v in inputs.items()}
    in_maps, meta = prep_host(**inputs)
    key = tuple(sorted(meta.items()))
    if key not in _CACHE:
        _CACHE[key] = build(meta)
    nc = _CACHE[key]
    res = run_bass_kernel_spmd(nc, in_maps, core_ids=list(range(M)),
                               trace=bool(int(os.environ.get("KERNEL_TRACE", "0"))))
    if getattr(res, "exec_time_ns", None) is not None:
        print(f"HW exec time: {res.exec_time_ns} ns")
    return np.asarray(res.results[0]["out"]).astype(np.float32)



# revision 3
# speedup vs baseline: 2.4694x; 2.4694x over previous
"""Distributed Trainium2 kernel for a GATv2 layer + BN + global-mean-pool + classifier.

Math (reference, heads=1):
    xl = x@Wl + bl ; xr = x@Wr + br
    logit_e = att . leaky_relu(xl[src_e] + xr[dst_e], 0.2)
    a_e     = segment_softmax(logit_e over dst)
    out_i   = sum_{e: dst=i} a_e * xl[src_e] ; out = out + bias1
    h       = BN(out) ; g = mean_i h ; y = softmax(g@Wc + bc)

Only the global mean over nodes matters, so per-node outputs never
materialize:
    y = softmax( ((S/N)*A + B) @ Wc + bc ),  S = sum_e a_e * xl[src_e],
    A = gamma/sqrt(var+eps), B = (bias1 - mu)*A + beta.

Attention weights v = att are folded into the tables host-side:
    v_f * lrelu(z_f) = sign_f * lrelu(|v_f| z_f)
with features permuted so positive-sign features occupy columns [0,PP).

Layout: per core, nodes sorted by in-degree (desc) and processed 128 per
batch, one node per partition; a node's in-edges occupy D consecutive
slot-columns of its partition row (D = max degree in batch; batches with
equal D are grouped into chunks).  Per edge ONE dma_gather fetches the
packed pair row of xl[src] (int16 indices address node pairs); xr[dst] is
a stride-0 broadcast of the node's own row; the self-loop edge is computed
straight from the local tables (no gather).  Segment softmax is a plain
row-reduce per batch.  The weighted sum uses sum_e w_e z_e - sum_d xr_d
(softmax weights sum to 1 per node), accumulated per-column into a
[128, F] accumulator, finished with one ones-matmul + AllReduce + head.
"""

import os

import ml_dtypes
import numpy as np

import concourse.bass as bass
import concourse.bacc as bacc
import concourse.mybir as mybir
import concourse.tile as tile

M = 8  # cores
F = 128
NCLS = 5
BN_EPS = 1e-5
NPCR = 6250     # real nodes per core
NB = 49         # batches of 128 nodes (6272 padded)
NPC = NB * 128
NG = M * NPC
CAP = 32        # max slot-columns per chunk (SBUF budget)
NBC = 16        # max batches per chunk

BF16 = ml_dtypes.bfloat16


def _wrap_idx(seq):
    """[n] int array -> [128, n//16] int16 wrap layout (16-partition groups,
    replicated across the 8 gpsimd cores)."""
    n = seq.shape[0]
    assert n % 16 == 0
    w = seq.reshape(n // 16, 16).T.astype(np.int16)
    return np.tile(w, (8, 1))


def prep_host(x, edge_index, Wl, bl, Wr, br, att, bias1,
              bn_gamma, bn_beta, bn_mean, bn_var, Wc, bc):
    N = x.shape[0]
    assert N == NPCR * M
    src = np.asarray(edge_index[0], np.int64)
    dst = np.asarray(edge_index[1], np.int64)

    # ---- attention folding ----
    v = np.asarray(att[0], np.float64)
    posm = v >= 0
    perm = np.argsort(~posm, kind="stable")
    PP = int(posm.sum())
    assert 0 < PP < F, f"degenerate attention sign split PP={PP}"
    absv = np.abs(v[perm])
    Wg_l = (Wl[:, perm] * absv[None, :]).astype(np.float32)
    bg_l = (bl[perm] * absv).astype(np.float32)
    Wg_r = (Wr[:, perm] * absv[None, :]).astype(np.float32)
    bg_r = (br[perm] * absv).astype(np.float32)

    # ---- per-core degree-sorted node order ----
    deg = np.bincount(dst, minlength=N)  # in-degree excluding self loop
    rank = np.zeros(N, np.int64)         # node -> global padded rank
    xT = np.zeros((M, 128, NPC), BF16)
    smask = np.zeros((M, 128, NB), np.float32)  # real-node mask [p, b]
    Dbs = np.zeros((M, NB), np.int64)
    for k in range(M):
        lo = k * NPCR
        dk = deg[lo:lo + NPCR]
        order = np.argsort(-dk, kind="stable")     # rank -> local node
        rank[lo + order] = k * NPC + np.arange(NPCR)
        xk = np.zeros((NPC, F), np.float32)
        xk[:NPCR] = x[lo + order]
        xT[k] = np.ascontiguousarray(xk.T.astype(BF16))
        r = np.arange(NPC)
        smask[k] = ((r % 128) * 0 + (r < NPCR)).astype(np.float32) \
            .reshape(NB, 128).T
        Dbs[k] = np.concatenate([np.sort(dk)[::-1], np.zeros(NPC - NPCR,
                                np.int64)]).reshape(NB, 128).max(axis=1)

    # per-core chunk schedules must be IDENTICAL (SPMD one program).
    # Use the max D over cores for each batch index.
    Dmax_b = Dbs.max(axis=0)           # [NB] non-increasing? per-core sorted
    Dmax_b = np.maximum.accumulate(Dmax_b[::-1])[::-1]  # enforce non-increasing
    chunks = []   # (b0, nb_c, D, coloff)
    coloff = 0
    b = 0
    while b < NB:
        D = int(Dmax_b[b])
        e = b
        while e < NB and int(Dmax_b[e]) == D:
            e += 1
        run = e - b
        step = max(1, min(NBC, (CAP // D) if D > 0 else NBC))
        while b < e:
            nb_c = min(step, e - b)
            chunks.append((b, nb_c, D, coloff))
            coloff += nb_c * D
            b += nb_c
    TC = coloff  # total gathered columns
    TCpad = ((TC + 1 + 7) // 8) * 8  # pad idx width to mult of 8 cols

    # ---- per-core slot tables ----
    # CSR of in-edges by dst, in rank order
    iP = np.zeros((M, 128, TCpad), np.int64)
    par = np.zeros((M, 128, TCpad), np.float32)
    pmask = np.zeros((M, 128, TCpad), np.float32)
    srcrow = rank[src]
    for k in range(M):
        lo = k * NPCR
        sel = (dst >= lo) & (dst < lo + NPCR)
        d_r = rank[dst[sel]] - k * NPC        # local rank of dst
        s_r = srcrow[sel]                     # global padded rank of src
        o = np.argsort(d_r, kind="stable")
        d_r = d_r[o]
        s_r = s_r[o]
        cnt = np.bincount(d_r, minlength=NPC)
        starts = np.concatenate([[0], np.cumsum(cnt)])
        # slot (p, col) for chunk (b0, nb, D): col = coloff + bi*D + d
        # edge d of node rank (b0+bi)*128 + p
        pos_in_seg = np.arange(len(d_r)) - starts[d_r]
        bnode = d_r // 128
        pnode = d_r % 128
        # find chunk of bnode
        colbase = np.zeros(NB, np.int64)
        Dof = np.zeros(NB, np.int64)
        for (b0, nb_c, D, co) in chunks:
            for bi in range(nb_c):
                colbase[b0 + bi] = co + bi * D
                Dof[b0 + bi] = D
        assert (pos_in_seg < Dof[bnode]).all()
        cols = colbase[bnode] + pos_in_seg
        iP[k, pnode, cols] = s_r >> 1
        par[k, pnode, cols] = (s_r & 1).astype(np.float32)
        pmask[k, pnode, cols] = 1.0

    iP_w = np.stack([
        _wrap_idx(iP[k, :, :TCpad].T.reshape(-1)) for k in range(M)])

    # ---- head constants ----
    A = bn_gamma.astype(np.float64) / np.sqrt(bn_var.astype(np.float64) + BN_EPS)
    Ap = (A[perm] / (N * absv)).astype(np.float32).reshape(F, 1)
    Bp = ((bias1 - bn_mean).astype(np.float64) * A + bn_beta)[perm] \
        .astype(np.float32).reshape(F, 1)
    Wcp = Wc[perm, :].astype(np.float32)

    meta = dict(PP=PP, TC=TC, TCpad=TCpad, chunks=tuple(chunks))

    in_maps = []
    for k in range(M):
        in_maps.append({
            "xT": np.ascontiguousarray(xT[k]),
            "Wgl": Wg_l.astype(BF16),
            "bgl": bg_l.reshape(1, F).astype(BF16),
            "Wgr": Wg_r.astype(BF16),
            "bgr": bg_r.reshape(1, F).astype(BF16),
            "iP": np.ascontiguousarray(iP_w[k]),
            "par": np.ascontiguousarray(par[k]),
            "pmask": np.ascontiguousarray(pmask[k]),
            "smask": np.ascontiguousarray(smask[k]),
            "smaskb": np.ascontiguousarray(smask[k].astype(BF16)),
            "Ap": Ap,
            "Bp": Bp,
            "Wcp": Wcp,
            "bc": bc.reshape(1, NCLS).astype(np.float32),
        })
    return in_maps, meta


def build(meta):
    PP, TC, TCpad = meta["PP"], meta["TC"], meta["TCpad"]
    chunks = meta["chunks"]
    CAPC = max(32, max(nb * D for (_, nb, D, _) in chunks))
    LW = (TCpad * 128) // 16

    dt = mybir.dt
    alu = mybir.AluOpType
    act = mybir.ActivationFunctionType
    rg = [list(range(M))]

    nc = bacc.Bacc("TRN2", target_bir_lowering=False, debug=False, num_devices=M)

    def p_in(name, shape, d):
        return nc.dram_tensor(name, shape, d, kind="ExternalInput").ap()

    xT = p_in("xT", [128, NPC], dt.bfloat16)
    Wgl = p_in("Wgl", [F, F], dt.bfloat16)
    bgl = p_in("bgl", [1, F], dt.bfloat16)
    Wgr = p_in("Wgr", [F, F], dt.bfloat16)
    bgr = p_in("bgr", [1, F], dt.bfloat16)
    iP = p_in("iP", [128, LW], dt.int16)
    par = p_in("par", [128, TCpad], dt.float32)
    pmask = p_in("pmask", [128, TCpad], dt.float32)
    smask = p_in("smask", [128, NB], dt.float32)
    smaskb = p_in("smaskb", [128, NB], dt.bfloat16)
    Ap = p_in("Ap", [F, 1], dt.float32)
    Bp = p_in("Bp", [F, 1], dt.float32)
    Wcp = p_in("Wcp", [F, NCLS], dt.float32)
    bc = p_in("bc", [1, NCLS], dt.float32)
    out = nc.dram_tensor("out", [1, NCLS], dt.float32, kind="ExternalOutput").ap()

    with tile.TileContext(nc) as tc:
        with (
            tc.tile_pool(name="dram", bufs=1, space="DRAM") as dpool,
            tc.tile_pool(name="sbp", bufs=1) as sbp,
            tc.tile_pool(name="sbw", bufs=2) as sbw,
            tc.tile_pool(name="ps2", bufs=2, space="PSUM") as pp,
            tc.tile_pool(name="ps1", bufs=1, space="PSUM") as pp1,
        ):
            xg_loc = dpool.tile([NPC, F], dt.bfloat16)
            xg_full = dpool.tile([NG, F], dt.bfloat16, addr_space="Shared")
            part_loc = dpool.tile([1, F], dt.float32)
            pooled = dpool.tile([1, F], dt.float32, addr_space="Shared")

            # ---- persistent SBUF ----
            xT_sb = sbp.tile([128, NPC], dt.bfloat16)
            nc.sync.dma_start(xT_sb[:], xT)
            wt = {}
            for nm, apin, sh in (("Wgl", Wgl, [F, F]), ("bgl", bgl, [1, F]),
                                 ("Wgr", Wgr, [F, F]), ("bgr", bgr, [1, F])):
                tl = sbp.tile(sh, dt.bfloat16, tag=nm)
                nc.sync.dma_start(tl[:], apin)
                wt[nm] = tl
            ones_sb = sbp.tile([1, F], dt.bfloat16)
            nc.vector.memset(ones_sb[:], 1.0)
            ones_f = sbp.tile([128, 1], dt.float32)
            nc.vector.memset(ones_f[:], 1.0)

            iP_sb = sbp.tile([128, LW], dt.int16)
            nc.sync.dma_start(iP_sb[:], iP)
            par_sb = sbp.tile([128, TCpad], dt.float32)
            nc.sync.dma_start(par_sb[:], par)
            pm_sb = sbp.tile([128, TCpad], dt.float32)
            nc.sync.dma_start(pm_sb[:], pmask)
            sm_sb = sbp.tile([128, NB], dt.float32)
            nc.sync.dma_start(sm_sb[:], smask)
            smb_sb = sbp.tile([128, NB], dt.bfloat16)
            nc.sync.dma_start(smb_sb[:], smaskb)

            xgl_sb = sbp.tile([128, NB * F], dt.bfloat16)
            xgr_sb = sbp.tile([128, NB * F], dt.bfloat16)
            vacc = sbp.tile([128, F], dt.float32)
            nc.vector.memset(vacc[:], 0.0)

            # ---- stage A: node tables (nodes in degree-sorted order) ----
            xrs = pp1.tile([F, 1], dt.float32, tag="xrs")
            for ci in range(NB):
                lhs = xT_sb[:, 128 * ci:128 * (ci + 1)]
                for wn, bn_, dstt in (("Wgl", "bgl", xgl_sb),
                                      ("Wgr", "bgr", xgr_sb)):
                    ps = pp.tile([128, F], dt.float32, tag="psA")
                    nc.tensor.matmul(ps[:], lhsT=lhs, rhs=wt[wn][:],
                                     start=True, stop=False)
                    nc.tensor.matmul(ps[:], lhsT=ones_sb[:], rhs=wt[bn_][:],
                                     start=False, stop=True)
                    sl = dstt[:, F * ci:F * (ci + 1)]
                    nc.vector.tensor_copy(sl, ps[:])
                    if wn == "Wgl":
                        rows = slice(128 * ci, 128 * (ci + 1))
                        nc.sync.dma_start(xg_loc[rows, :], sl)
                    else:
                        # xr_sum += sum over real rows of this chunk
                        nc.tensor.matmul(xrs[:], lhsT=sl,
                                         rhs=smb_sb[:, ci:ci + 1],
                                         start=(ci == 0), stop=(ci == NB - 1))

            nc.gpsimd.collective_compute(
                "AllGather", mybir.AluOpType.bypass, replica_groups=rg,
                ins=[xg_loc.opt()], outs=[xg_full.opt()])
            tab_pair = xg_full[:].rearrange("(a two) f -> a (two f)", two=2)

            # ---- main loop: one chunk = nb_c batches of equal D ----
            for (b0, nb_c, D, coloff) in chunks:
                cols = nb_c * D
                nbF = nb_c * F
                bsl = slice(b0 * F, (b0 + nb_c) * F)
                # --- self-loop columns from local tables ---
                z0 = sbw.tile([128, NBC * F], dt.bfloat16, tag="z0")
                nc.vector.tensor_tensor(out=z0[:, :nbF], in0=xgl_sb[:, bsl],
                                        in1=xgr_sb[:, bsl], op=alu.add)
                za0 = sbw.tile([128, NBC * F], dt.bfloat16, tag="za0")
                nc.scalar.activation(za0[:, :nbF], z0[:, :nbF], act.Abs,
                                     scale=0.4)
                nc.vector.scalar_tensor_tensor(
                    out=za0[:, :nbF], in0=z0[:, :nbF], scalar=0.6,
                    in1=za0[:, :nbF], op0=alu.mult, op1=alu.add)
                m0 = za0[:, :nbF].rearrange("p (b f) -> p b f", f=F)
                l0p = sbw.tile([128, NBC], dt.float32, tag="l0p")
                l0n = sbw.tile([128, NBC], dt.float32, tag="l0n")
                nc.vector.tensor_reduce(l0p[:, :nb_c], m0[:, :, 0:PP],
                                        axis=mybir.AxisListType.X, op=alu.add)
                nc.vector.tensor_reduce(l0n[:, :nb_c], m0[:, :, PP:F],
                                        axis=mybir.AxisListType.X, op=alu.add)
                nc.vector.tensor_tensor(out=l0p[:, :nb_c], in0=l0p[:, :nb_c],
                                        in1=l0n[:, :nb_c], op=alu.subtract)
                E0 = sbw.tile([128, NBC], dt.float32, tag="E0")
                nc.scalar.activation(E0[:, :nb_c], l0p[:, :nb_c], act.Exp)

                if cols > 0:
                    csl = slice(coloff, coloff + cols)
                    gp = sbw.tile([128, CAPC * 2 * F], dt.bfloat16, tag="gp",
                                  bufs=3)
                    nc.gpsimd.dma_gather(
                        out_ap=gp[:, :cols * 2 * F].rearrange(
                            "p (c f) -> p c f", f=2 * F),
                        in_ap=tab_pair,
                        idxs_ap=iP_sb[:, coloff * 8:(coloff + cols) * 8],
                        num_idxs=cols * 128, num_idxs_reg=cols * 128,
                        elem_size=2 * F, single_packet=False)
                    g3 = gp[:, :cols * 2 * F].rearrange("p (c f) -> p c f",
                                                        f=2 * F)
                    # select pair half:  G = G0 + par*(G1-G0)
                    gs = sbw.tile([128, CAPC * F], dt.bfloat16, tag="gs")
                    gs3 = gs[:, :cols * F].rearrange("p (c f) -> p c f", f=F)
                    nc.vector.tensor_tensor(out=gs3, in0=g3[:, :, F:2 * F],
                                            in1=g3[:, :, 0:F], op=alu.subtract)
                    par_b = par_sb[:, csl].rearrange(
                        "p (c one) -> p c one", one=1).to_broadcast(
                        (128, cols, F))
                    nc.vector.tensor_tensor(out=gs3, in0=gs3, in1=par_b,
                                            op=alu.mult)
                    nc.vector.tensor_tensor(out=gs3, in0=gs3,
                                            in1=g3[:, :, 0:F], op=alu.add)
                    # z = G + xr[dst]  (per batch: broadcast node row over D)
                    z = sbw.tile([128, CAPC * F], dt.bfloat16, tag="z")
                    for bi in range(nb_c):
                        zsl = z[:, bi * D * F:(bi + 1) * D * F].rearrange(
                            "p (c f) -> p c f", f=F)
                        gsl = gs[:, bi * D * F:(bi + 1) * D * F].rearrange(
                            "p (c f) -> p c f", f=F)
                        xr_b = xgr_sb[:, (b0 + bi) * F:(b0 + bi + 1) * F] \
                            .rearrange("p (one f) -> p one f", one=1) \
                            .to_broadcast((128, D, F))
                        nc.vector.tensor_tensor(out=zsl, in0=gsl, in1=xr_b,
                                                op=alu.add)
                    # lrelu via 0.6 z + 0.4|z|
                    za = sbw.tile([128, CAPC * F], dt.bfloat16, tag="za")
                    nc.scalar.activation(za[:, :cols * F], z[:, :cols * F],
                                         act.Abs, scale=0.4)
                    nc.vector.scalar_tensor_tensor(
                        out=za[:, :cols * F], in0=z[:, :cols * F], scalar=0.6,
                        in1=za[:, :cols * F], op0=alu.mult, op1=alu.add)
                    m3 = za[:, :cols * F].rearrange("p (c f) -> p c f", f=F)
                    lg = sbw.tile([128, CAPC], dt.float32, tag="lg")
                    lgn = sbw.tile([128, CAPC], dt.float32, tag="lgn")
                    nc.vector.tensor_reduce(lg[:, :cols], m3[:, :, 0:PP],
                                            axis=mybir.AxisListType.X,
                                            op=alu.add)
                    nc.vector.tensor_reduce(lgn[:, :cols], m3[:, :, PP:F],
                                            axis=mybir.AxisListType.X,
                                            op=alu.add)
                    nc.vector.tensor_tensor(out=lg[:, :cols], in0=lg[:, :cols],
                                            in1=lgn[:, :cols], op=alu.subtract)
                    E = sbw.tile([128, CAPC], dt.float32, tag="E")
                    nc.scalar.activation(E[:, :cols], lg[:, :cols], act.Exp)
                    nc.vector.tensor_tensor(out=E[:, :cols], in0=E[:, :cols],
                                            in1=pm_sb[:, csl], op=alu.mult)
                    den = sbw.tile([128, NBC], dt.float32, tag="den")
                    nc.vector.tensor_reduce(
                        den[:, :nb_c],
                        E[:, :cols].rearrange("p (b d) -> p b d", d=D),
                        axis=mybir.AxisListType.X, op=alu.add)
                    nc.vector.tensor_tensor(out=den[:, :nb_c],
                                            in0=den[:, :nb_c],
                                            in1=E0[:, :nb_c], op=alu.add)
                else:
                    den = E0

                rd = sbw.tile([128, NBC], dt.float32, tag="rd")
                nc.vector.reciprocal(rd[:, :nb_c], den[:, :nb_c])
                # w0 = E0 * rd * smask
                w0 = sbw.tile([128, NBC], dt.float32, tag="w0")
                nc.vector.tensor_tensor(out=w0[:, :nb_c], in0=E0[:, :nb_c],
                                        in1=rd[:, :nb_c], op=alu.mult)
                nc.vector.tensor_tensor(out=w0[:, :nb_c], in0=w0[:, :nb_c],
                                        in1=sm_sb[:, b0:b0 + nb_c],
                                        op=alu.mult)
                if cols > 0:
                    w = sbw.tile([128, CAPC], dt.float32, tag="w")
                    rd_b = rd[:, :nb_c].rearrange(
                        "p (b one) -> p b one", one=1).to_broadcast(
                        (128, nb_c, D))
                    nc.vector.tensor_tensor(
                        out=w[:, :cols].rearrange("p (b d) -> p b d", d=D),
                        in0=E[:, :cols].rearrange("p (b d) -> p b d", d=D),
                        in1=rd_b, op=alu.mult)
                    for t in range(cols):
                        nc.vector.scalar_tensor_tensor(
                            out=vacc[:], in0=z[:, t * F:(t + 1) * F],
                            scalar=w[:, t:t + 1], in1=vacc[:],
                            op0=alu.mult, op1=alu.add)
                for bi in range(nb_c):
                    nc.vector.scalar_tensor_tensor(
                        out=vacc[:], in0=z0[:, bi * F:(bi + 1) * F],
                        scalar=w0[:, bi:bi + 1], in1=vacc[:],
                        op0=alu.mult, op1=alu.add)

            # ---- tail:  S = sum_p vacc - xr_sum ; AllReduce ; head ----
            sps = pp1.tile([F, 1], dt.float32, tag="sps")
            nc.tensor.matmul(sps[:], lhsT=vacc[:], rhs=ones_f[:],
                             start=True, stop=True)
            xrs_sb = sbp.tile([F, 1], dt.float32)
            nc.vector.tensor_copy(xrs_sb[:], xrs[:])
            part_sb = sbp.tile([F, 1], dt.float32)
            nc.vector.tensor_tensor(out=part_sb[:], in0=sps[:], in1=xrs_sb[:],
                                    op=alu.subtract)
            nc.sync.dma_start(part_loc[:].rearrange("o f -> f o"), part_sb[:])

            nc.gpsimd.collective_compute(
                "AllReduce", alu.add, replica_groups=rg,
                ins=[part_loc.opt()], outs=[pooled.opt()])

            # ---- head ----
            pool_sb = sbp.tile([F, 1], dt.float32)
            nc.sync.dma_start(pool_sb[:], pooled[:].rearrange("o f -> f o"))
            Ap_sb = sbp.tile([F, 1], dt.float32)
            nc.sync.dma_start(Ap_sb[:], Ap)
            Bp_sb = sbp.tile([F, 1], dt.float32)
            nc.sync.dma_start(Bp_sb[:], Bp)
            Wc_sb = sbp.tile([F, NCLS], dt.float32)
            nc.sync.dma_start(Wc_sb[:], Wcp)
            bc_sb = sbp.tile([1, NCLS], dt.float32)
            nc.sync.dma_start(bc_sb[:], bc)
            h_sb = sbp.tile([F, 1], dt.float32)
            nc.vector.scalar_tensor_tensor(
                out=h_sb[:], in0=pool_sb[:], scalar=Ap_sb[:, 0:1], in1=Bp_sb[:],
                op0=alu.mult, op1=alu.add)
            one1 = sbp.tile([1, 1], dt.float32)
            nc.vector.memset(one1[:], 1.0)
            hp = pp1.tile([1, NCLS], dt.float32, tag="hp")
            nc.tensor.matmul(hp[:], lhsT=h_sb[:], rhs=Wc_sb[:], start=True,
                             stop=False)
            nc.tensor.matmul(hp[:], lhsT=one1[:], rhs=bc_sb[:], start=False,
                             stop=True)
            eh = sbp.tile([1, NCLS], dt.float32)
            nc.scalar.activation(eh[:], hp[:], act.Exp)
            den = sbp.tile([1, 1], dt.float32)
            nc.vector.tensor_reduce(den[:], eh[:], axis=mybir.AxisListType.X,
                                    op=alu.add)
            rden = sbp.tile([1, 1], dt.float32)
            nc.vector.reciprocal(rden[:], den[:])
            osb = sbp.tile([1, NCLS], dt.float32)
            nc.vector.tensor_scalar(out=osb[:], in0=eh[:], scalar1=rden[:, 0:1],
                                    scalar2=None, op0=alu.mult)
            nc.sync.dma_start(out, osb[:])

    nc.compile()
    return nc


# --------------------------------------------------------------------------
# public entry point
# --------------------------------------------------------------------------

_CACHE = {}


def _install_ntff_hook():
    """Provide antenv.axon_hooks + the ctypes NTFF hook when the image lacks
    them, so run_bass_kernel_spmd(trace=True) can capture exec_time_ns."""
    import contextlib
    import ctypes
    import sys
    import types

    try:
        import antenv.axon_hooks  # noqa: F401
        return
    except ImportError:
        pass
    try:
        import antenv
    except ImportError:
        return
    holder = [None]
    mod = types.ModuleType("antenv.axon_hooks")
    mod.set_axon_ntff_profile_hook = lambda h: holder.__setitem__(0, h)
    mod.get_axon_ntff_profile_hook = lambda: holder[0]
    sys.modules["antenv.axon_hooks"] = mod
    antenv.axon_hooks = mod

    so_path = "/opt/axon/libaxon_pjrt.so"
    if os.path.exists(so_path):
        lib = ctypes.CDLL(so_path)
        if hasattr(lib, "axon_start_nrt_profile"):
            lib.axon_start_nrt_profile.argtypes = [
                ctypes.POINTER(ctypes.c_int64), ctypes.c_size_t]
            lib.axon_start_nrt_profile.restype = ctypes.c_int64
            lib.axon_stop_nrt_profile.argtypes = [ctypes.c_char_p]
            lib.axon_stop_nrt_profile.restype = ctypes.c_int64

            @contextlib.contextmanager
            def _hook(output_dir, device_ids):
                import jax
                jax.devices()
                if device_ids:
                    ids = (ctypes.c_int64 * len(device_ids))(*device_ids)
                    rc = lib.axon_start_nrt_profile(ids, len(device_ids))
                else:
                    rc = lib.axon_start_nrt_profile(None, 0)
                if rc != 0:
                    raise RuntimeError(f"axon_start_nrt_profile rc={rc}")
                try:
                    yield
                finally:
                    n = lib.axon_stop_nrt_profile(str(output_dir).encode())
                    print(f"ntff profile: {n} file(s) -> {output_dir}")

            mod.set_axon_ntff_profile_hook(_hook)

    import concourse.bass_utils as bu
    bu.upload_artifacts = lambda tmpdir: "local://" + str(tmpdir)


def kernel(**inputs):
    from concourse.bass_utils import run_bass_kernel_spmd

    if bool(int(os.environ.get("KERNEL_TRACE", "0"))):
        _install_ntff_hook()
    inputs = {k: np.asarray(v) for k, v in inputs.items()}
    in_maps, meta = prep_host(**inputs)
    key = (meta["PP"], meta["TC"], meta["chunks"])
    if key not in _CACHE:
        _CACHE[key] = build(meta)
    nc = _CACHE[key]
    res = run_bass_kernel_spmd(nc, in_maps, core_ids=list(range(M)),
                               trace=bool(int(os.environ.get("KERNEL_TRACE", "0"))))
    if getattr(res, "exec_time_ns", None) is not None:
        print(f"HW exec time: {res.exec_time_ns} ns")
    return np.asarray(res.results[0]["out"]).astype(np.float32)


# revision 5
# speedup vs baseline: 2.5469x; 1.0314x over previous
"""Distributed Trainium2 kernel for a GATv2 layer + BN + global-mean-pool + classifier.

Math (reference, heads=1):
    xl = x@Wl + bl ; xr = x@Wr + br
    logit_e = att . leaky_relu(xl[src_e] + xr[dst_e], 0.2)
    a_e     = segment_softmax(logit_e over dst)
    out_i   = sum_{e: dst=i} a_e * xl[src_e] ; out = out + bias1
    h       = BN(out) ; g = mean_i h ; y = softmax(g@Wc + bc)

Only the global mean over nodes matters, so per-node outputs never
materialize:
    y = softmax( ((S/N)*A + B) @ Wc + bc ),  S = sum_e a_e * xl[src_e],
    A = gamma/sqrt(var+eps), B = (bias1 - mu)*A + beta.

Attention weights v = att are folded into the tables host-side:
    v_f * lrelu(z_f) = sign_f * lrelu(|v_f| z_f)
with features permuted so positive-sign features occupy columns [0,PP).

Layout: per core, nodes sorted by in-degree (desc) and processed 128 per
batch, one node per partition; a node's in-edges occupy D consecutive
slot-columns of its partition row (D = max degree in batch; batches with
equal D are grouped into chunks).  Per edge ONE dma_gather fetches the
packed pair row of xl[src] (int16 indices address node pairs); xr[dst] is
a stride-0 broadcast of the node's own row; the self-loop edge is computed
straight from the local tables (no gather).  Segment softmax is a plain
row-reduce per batch.  The weighted sum uses sum_e w_e z_e - sum_d xr_d
(softmax weights sum to 1 per node), accumulated per-column into a
[128, F] accumulator, finished with one ones-matmul + AllReduce + head.
"""

import os

import ml_dtypes
import numpy as np

import concourse.bass as bass
import concourse.bacc as bacc
import concourse.mybir as mybir
import concourse.tile as tile

M = 8  # cores
F = 128
NCLS = 5
BN_EPS = 1e-5
NPCR = 6250     # real nodes per core
NB = 49         # batches of 128 nodes (6272 padded)
NPC = NB * 128
NG = M * NPC
CAP = 32        # max slot-columns per chunk (SBUF budget)
NBC = 16        # max batches per chunk

BF16 = ml_dtypes.bfloat16


def _wrap_idx(seq):
    """[n] int array -> [128, n//16] int16 wrap layout (16-partition groups,
    replicated across the 8 gpsimd cores)."""
    n = seq.shape[0]
    assert n % 16 == 0
    w = seq.reshape(n // 16, 16).T.astype(np.int16)
    return np.tile(w, (8, 1))


def prep_host(x, edge_index, Wl, bl, Wr, br, att, bias1,
              bn_gamma, bn_beta, bn_mean, bn_var, Wc, bc):
    N = x.shape[0]
    assert N == NPCR * M
    src = np.asarray(edge_index[0], np.int64)
    dst = np.asarray(edge_index[1], np.int64)

    # ---- attention folding ----
    v = np.asarray(att[0], np.float64)
    posm = v >= 0
    perm = np.argsort(~posm, kind="stable")
    PP = int(posm.sum())
    assert 0 < PP < F, f"degenerate attention sign split PP={PP}"
    absv = np.abs(v[perm])
    Wg_l = (Wl[:, perm] * absv[None, :]).astype(np.float32)
    bg_l = (bl[perm] * absv).astype(np.float32)
    Wg_r = (Wr[:, perm] * absv[None, :]).astype(np.float32)
    bg_r = (br[perm] * absv).astype(np.float32)

    # ---- per-core degree-sorted node order ----
    deg = np.bincount(dst, minlength=N)  # in-degree excluding self loop
    rank = np.zeros(N, np.int64)         # node -> global padded rank
    xT = np.zeros((M, 128, NPC), BF16)
    smask = np.zeros((M, 128, NB), np.float32)  # real-node mask [p, b]
    Dbs = np.zeros((M, NB), np.int64)
    for k in range(M):
        lo = k * NPCR
        dk = deg[lo:lo + NPCR]
        order = np.argsort(-dk, kind="stable")     # rank -> local node
        rank[lo + order] = k * NPC + np.arange(NPCR)
        xk = np.zeros((NPC, F), np.float32)
        xk[:NPCR] = x[lo + order]
        xT[k] = np.ascontiguousarray(xk.T.astype(BF16))
        r = np.arange(NPC)
        smask[k] = ((r % 128) * 0 + (r < NPCR)).astype(np.float32) \
            .reshape(NB, 128).T
        Dbs[k] = np.concatenate([np.sort(dk)[::-1], np.zeros(NPC - NPCR,
                                np.int64)]).reshape(NB, 128).max(axis=1)

    # per-core chunk schedules must be IDENTICAL (SPMD one program).
    # Use the max D over cores for each batch index.
    Dmax_b = Dbs.max(axis=0)           # [NB] non-increasing? per-core sorted
    Dmax_b = np.maximum.accumulate(Dmax_b[::-1])[::-1]  # enforce non-increasing
    chunks = []   # (b0, nb_c, D, coloff)
    coloff = 0
    b = 0
    while b < NB:
        D = int(Dmax_b[b])
        e = b
        while e < NB and int(Dmax_b[e]) == D:
            e += 1
        run = e - b
        step = max(1, min(NBC, (CAP // D) if D > 0 else NBC))
        while b < e:
            nb_c = min(step, e - b)
            chunks.append((b, nb_c, D, coloff))
            coloff += nb_c * D
            b += nb_c
    TC = coloff  # total gathered columns
    TCpad = ((TC + 1 + 7) // 8) * 8  # pad idx width to mult of 8 cols

    # ---- per-core slot tables ----
    # CSR of in-edges by dst, in rank order
    iP = np.zeros((M, 128, TCpad), np.int64)
    par = np.zeros((M, 128, TCpad), np.float32)
    pmask = np.zeros((M, 128, TCpad), np.float32)
    srcrow = rank[src]
    for k in range(M):
        lo = k * NPCR
        sel = (dst >= lo) & (dst < lo + NPCR)
        d_r = rank[dst[sel]] - k * NPC        # local rank of dst
        s_r = srcrow[sel]                     # global padded rank of src
        o = np.argsort(d_r, kind="stable")
        d_r = d_r[o]
        s_r = s_r[o]
        cnt = np.bincount(d_r, minlength=NPC)
        starts = np.concatenate([[0], np.cumsum(cnt)])
        # slot (p, col) for chunk (b0, nb, D): col = coloff + bi*D + d
        # edge d of node rank (b0+bi)*128 + p
        pos_in_seg = np.arange(len(d_r)) - starts[d_r]
        bnode = d_r // 128
        pnode = d_r % 128
        # find chunk of bnode
        colbase = np.zeros(NB, np.int64)
        Dof = np.zeros(NB, np.int64)
        for (b0, nb_c, D, co) in chunks:
            for bi in range(nb_c):
                colbase[b0 + bi] = co + bi * D
                Dof[b0 + bi] = D
        assert (pos_in_seg < Dof[bnode]).all()
        cols = colbase[bnode] + pos_in_seg
        iP[k, pnode, cols] = s_r >> 1
        par[k, pnode, cols] = (s_r & 1).astype(np.float32)
        pmask[k, pnode, cols] = 1.0

    iP_w = np.stack([
        _wrap_idx(iP[k, :, :TCpad].T.reshape(-1)) for k in range(M)])

    # ---- head constants ----
    A = bn_gamma.astype(np.float64) / np.sqrt(bn_var.astype(np.float64) + BN_EPS)
    Ap = (A[perm] / (N * absv)).astype(np.float32).reshape(F, 1)
    Bp = ((bias1 - bn_mean).astype(np.float64) * A + bn_beta)[perm] \
        .astype(np.float32).reshape(F, 1)
    Wcp = Wc[perm, :].astype(np.float32)

    meta = dict(PP=PP, TC=TC, TCpad=TCpad, chunks=tuple(chunks))

    in_maps = []
    for k in range(M):
        in_maps.append({
            "xT": np.ascontiguousarray(xT[k]),
            "Wgl": Wg_l.astype(BF16),
            "bgl": bg_l.reshape(1, F).astype(BF16),
            "Wgr": Wg_r.astype(BF16),
            "bgr": bg_r.reshape(1, F).astype(BF16),
            "iP": np.ascontiguousarray(iP_w[k]),
            "par": np.ascontiguousarray(par[k]),
            "pmask": np.ascontiguousarray(pmask[k]),
            "smask": np.ascontiguousarray(smask[k]),
            "smaskb": np.ascontiguousarray(smask[k].astype(BF16)),
            "Ap": Ap,
            "Bp": Bp,
            "Wcp": Wcp,
            "bc": bc.reshape(1, NCLS).astype(np.float32),
        })
    return in_maps, meta


def build(meta):
    PP, TC, TCpad = meta["PP"], meta["TC"], meta["TCpad"]
    chunks = meta["chunks"]
    CAPC = max(32, max(nb * D for (_, nb, D, _) in chunks))
    LW = (TCpad * 128) // 16

    dt = mybir.dt
    alu = mybir.AluOpType
    act = mybir.ActivationFunctionType
    rg = [list(range(M))]

    nc = bacc.Bacc("TRN2", target_bir_lowering=False, debug=False, num_devices=M)

    def p_in(name, shape, d):
        return nc.dram_tensor(name, shape, d, kind="ExternalInput").ap()

    xT = p_in("xT", [128, NPC], dt.bfloat16)
    Wgl = p_in("Wgl", [F, F], dt.bfloat16)
    bgl = p_in("bgl", [1, F], dt.bfloat16)
    Wgr = p_in("Wgr", [F, F], dt.bfloat16)
    bgr = p_in("bgr", [1, F], dt.bfloat16)
    iP = p_in("iP", [128, LW], dt.int16)
    par = p_in("par", [128, TCpad], dt.float32)
    pmask = p_in("pmask", [128, TCpad], dt.float32)
    smask = p_in("smask", [128, NB], dt.float32)
    smaskb = p_in("smaskb", [128, NB], dt.bfloat16)
    Ap = p_in("Ap", [F, 1], dt.float32)
    Bp = p_in("Bp", [F, 1], dt.float32)
    Wcp = p_in("Wcp", [F, NCLS], dt.float32)
    bc = p_in("bc", [1, NCLS], dt.float32)
    out = nc.dram_tensor("out", [1, NCLS], dt.float32, kind="ExternalOutput").ap()

    with tile.TileContext(nc) as tc:
        with (
            tc.tile_pool(name="dram", bufs=1, space="DRAM") as dpool,
            tc.tile_pool(name="sbp", bufs=1) as sbp,
            tc.tile_pool(name="sbw", bufs=2) as sbw,
            tc.tile_pool(name="ps2", bufs=2, space="PSUM") as pp,
            tc.tile_pool(name="ps1", bufs=1, space="PSUM") as pp1,
        ):
            xg_loc = dpool.tile([NPC, F], dt.bfloat16)
            xg_full = dpool.tile([NG, F], dt.bfloat16, addr_space="Shared")
            part_loc = dpool.tile([1, F], dt.float32)
            pooled = dpool.tile([1, F], dt.float32, addr_space="Shared")

            # ---- persistent SBUF ----
            xT_sb = sbp.tile([128, NPC], dt.bfloat16)
            nc.sync.dma_start(xT_sb[:], xT)
            wt = {}
            for nm, apin, sh in (("Wgl", Wgl, [F, F]), ("bgl", bgl, [1, F]),
                                 ("Wgr", Wgr, [F, F]), ("bgr", bgr, [1, F])):
                tl = sbp.tile(sh, dt.bfloat16, tag=nm)
                nc.sync.dma_start(tl[:], apin)
                wt[nm] = tl
            ones_sb = sbp.tile([1, F], dt.bfloat16)
            nc.vector.memset(ones_sb[:], 1.0)
            ones_f = sbp.tile([128, 1], dt.float32)
            nc.vector.memset(ones_f[:], 1.0)

            iP_sb = sbp.tile([128, LW], dt.int16)
            nc.sync.dma_start(iP_sb[:], iP)
            par_sb = sbp.tile([128, TCpad], dt.float32)
            nc.sync.dma_start(par_sb[:], par)
            pm_sb = sbp.tile([128, TCpad], dt.float32)
            nc.sync.dma_start(pm_sb[:], pmask)
            sm_sb = sbp.tile([128, NB], dt.float32)
            nc.sync.dma_start(sm_sb[:], smask)
            smb_sb = sbp.tile([128, NB], dt.bfloat16)
            nc.sync.dma_start(smb_sb[:], smaskb)

            xgl_sb = sbp.tile([128, NB * F], dt.bfloat16)
            xgr_sb = sbp.tile([128, NB * F], dt.bfloat16)

            # ---- stage A: node tables (nodes in degree-sorted order) ----
            xrs = pp1.tile([F, 1], dt.float32, tag="xrs")
            for ci in range(NB):
                lhs = xT_sb[:, 128 * ci:128 * (ci + 1)]
                for wn, bn_, dstt in (("Wgl", "bgl", xgl_sb),
                                      ("Wgr", "bgr", xgr_sb)):
                    ps = pp.tile([128, F], dt.float32, tag="psA")
                    nc.tensor.matmul(ps[:], lhsT=lhs, rhs=wt[wn][:],
                                     start=True, stop=False)
                    nc.tensor.matmul(ps[:], lhsT=ones_sb[:], rhs=wt[bn_][:],
                                     start=False, stop=True)
                    sl = dstt[:, F * ci:F * (ci + 1)]
                    nc.vector.tensor_copy(sl, ps[:])
                    if wn == "Wgl":
                        rows = slice(128 * ci, 128 * (ci + 1))
                        nc.sync.dma_start(xg_loc[rows, :], sl)
                    else:
                        # xr_sum += sum over real rows of this chunk
                        nc.tensor.matmul(xrs[:], lhsT=sl,
                                         rhs=smb_sb[:, ci:ci + 1],
                                         start=(ci == 0), stop=(ci == NB - 1))

            nc.gpsimd.collective_compute(
                "AllGather", mybir.AluOpType.bypass, replica_groups=rg,
                ins=[xg_loc.opt()], outs=[xg_full.opt()])
            tab_pair = xg_full[:].rearrange("(a two) f -> a (two f)", two=2)

            # ---- main loop: one chunk = nb_c batches of equal D ----
            pacc = pp1.tile([F, 1], dt.float32, tag="pacc")
            first_mm = [True]
            nmm = sum(2 * nb * D + nb for (_, nb, D, _) in chunks)
            mmleft = [nmm]

            def acc_mm(lhsT, rhs):
                nc.tensor.matmul(pacc[:], lhsT=lhsT, rhs=rhs,
                                 start=first_mm[0],
                                 stop=(mmleft[0] == 1))
                first_mm[0] = False
                mmleft[0] -= 1

            for (b0, nb_c, D, coloff) in chunks:
                cols = nb_c * D
                nbF = nb_c * F
                bsl = slice(b0 * F, (b0 + nb_c) * F)
                # --- self-loop columns from local tables ---
                z0 = sbw.tile([128, NBC * F], dt.bfloat16, tag="z0", bufs=3)
                nc.vector.tensor_tensor(out=z0[:, :nbF], in0=xgl_sb[:, bsl],
                                        in1=xgr_sb[:, bsl], op=alu.add)
                za0 = sbw.tile([128, NBC * F], dt.bfloat16, tag="za0")
                nc.scalar.activation(za0[:, :nbF], z0[:, :nbF], act.Lrelu,
                                     alpha=0.2)
                m0 = za0[:, :nbF].rearrange("p (b f) -> p b f", f=F)
                l0p = sbw.tile([128, NBC], dt.float32, tag="l0p")
                l0n = sbw.tile([128, NBC], dt.float32, tag="l0n")
                nc.vector.tensor_reduce(l0p[:, :nb_c], m0[:, :, 0:PP],
                                        axis=mybir.AxisListType.X, op=alu.add)
                nc.vector.tensor_reduce(l0n[:, :nb_c], m0[:, :, PP:F],
                                        axis=mybir.AxisListType.X, op=alu.add)
                nc.vector.tensor_tensor(out=l0p[:, :nb_c], in0=l0p[:, :nb_c],
                                        in1=l0n[:, :nb_c], op=alu.subtract)
                E0 = sbw.tile([128, NBC], dt.float32, tag="E0")
                nc.scalar.activation(E0[:, :nb_c], l0p[:, :nb_c], act.Exp)

                if cols > 0:
                    csl = slice(coloff, coloff + cols)
                    gp = sbw.tile([128, CAPC * 2 * F], dt.bfloat16, tag="gp",
                                  bufs=3)
                    nc.gpsimd.dma_gather(
                        out_ap=gp[:, :cols * 2 * F].rearrange(
                            "p (c f) -> p c f", f=2 * F),
                        in_ap=tab_pair,
                        idxs_ap=iP_sb[:, coloff * 8:(coloff + cols) * 8],
                        num_idxs=cols * 128, num_idxs_reg=cols * 128,
                        elem_size=2 * F, single_packet=False)
                    # z = gp + xr[dst] on BOTH pair halves
                    # (per batch: broadcast node row over 2D F-columns)
                    z = sbw.tile([128, CAPC * 2 * F], dt.bfloat16, tag="z",
                                 bufs=3)
                    for bi in range(nb_c):
                        sl2 = slice(bi * D * 2 * F, (bi + 1) * D * 2 * F)
                        xr_b = xgr_sb[:, (b0 + bi) * F:(b0 + bi + 1) * F] \
                            .rearrange("p (one f) -> p one f", one=1) \
                            .to_broadcast((128, 2 * D, F))
                        nc.vector.tensor_tensor(
                            out=z[:, sl2].rearrange("p (c f) -> p c f", f=F),
                            in0=gp[:, sl2].rearrange("p (c f) -> p c f", f=F),
                            in1=xr_b, op=alu.add)
                    za = sbw.tile([128, CAPC * 2 * F], dt.bfloat16, tag="za")
                    nc.scalar.activation(za[:, :cols * 2 * F],
                                         z[:, :cols * 2 * F], act.Lrelu,
                                         alpha=0.2)
                    m3 = za[:, :cols * 2 * F].rearrange("p (c f) -> p c f",
                                                        f=2 * F)
                    lgA = sbw.tile([128, CAPC], dt.float32, tag="lgA")
                    lgn = sbw.tile([128, CAPC], dt.float32, tag="lgn")
                    lgB = sbw.tile([128, CAPC], dt.float32, tag="lgB")
                    lgn1 = sbw.tile([128, CAPC], dt.float32, tag="lgn1")
                    nc.vector.tensor_reduce(lgA[:, :cols], m3[:, :, 0:PP],
                                            axis=mybir.AxisListType.X,
                                            op=alu.add)
                    nc.vector.tensor_reduce(lgn[:, :cols], m3[:, :, PP:F],
                                            axis=mybir.AxisListType.X,
                                            op=alu.add)
                    nc.vector.tensor_reduce(lgB[:, :cols], m3[:, :, F:F + PP],
                                            axis=mybir.AxisListType.X,
                                            op=alu.add)
                    nc.vector.tensor_reduce(lgn1[:, :cols],
                                            m3[:, :, F + PP:2 * F],
                                            axis=mybir.AxisListType.X,
                                            op=alu.add)
                    # logit = A + par*(B-A),  A = lgA-lgn, B = lgB-lgn1
                    nc.vector.tensor_tensor(out=lgA[:, :cols],
                                            in0=lgA[:, :cols],
                                            in1=lgn[:, :cols], op=alu.subtract)
                    nc.vector.tensor_tensor(out=lgB[:, :cols],
                                            in0=lgB[:, :cols],
                                            in1=lgn1[:, :cols],
                                            op=alu.subtract)
                    nc.vector.tensor_tensor(out=lgB[:, :cols],
                                            in0=lgB[:, :cols],
                                            in1=lgA[:, :cols], op=alu.subtract)
                    nc.vector.tensor_tensor(out=lgB[:, :cols],
                                            in0=lgB[:, :cols],
                                            in1=par_sb[:, csl], op=alu.mult)
                    nc.vector.tensor_tensor(out=lgA[:, :cols],
                                            in0=lgA[:, :cols],
                                            in1=lgB[:, :cols], op=alu.add)
                    E = sbw.tile([128, CAPC], dt.float32, tag="E")
                    nc.scalar.activation(E[:, :cols], lgA[:, :cols], act.Exp)
                    nc.vector.tensor_tensor(out=E[:, :cols], in0=E[:, :cols],
                                            in1=pm_sb[:, csl], op=alu.mult)
                    den = sbw.tile([128, NBC], dt.float32, tag="den")
                    nc.vector.tensor_reduce(
                        den[:, :nb_c],
                        E[:, :cols].rearrange("p (b d) -> p b d", d=D),
                        axis=mybir.AxisListType.X, op=alu.add)
                    nc.vector.tensor_tensor(out=den[:, :nb_c],
                                            in0=den[:, :nb_c],
                                            in1=E0[:, :nb_c], op=alu.add)
                else:
                    den = E0

                rd = sbw.tile([128, NBC], dt.float32, tag="rd")
                nc.vector.reciprocal(rd[:, :nb_c], den[:, :nb_c])
                # w0 = E0 * rd * smask  (bf16 for the PE)
                w0 = sbw.tile([128, NBC], dt.float32, tag="w0")
                nc.vector.tensor_tensor(out=w0[:, :nb_c], in0=E0[:, :nb_c],
                                        in1=rd[:, :nb_c], op=alu.mult)
                nc.vector.tensor_tensor(out=w0[:, :nb_c], in0=w0[:, :nb_c],
                                        in1=sm_sb[:, b0:b0 + nb_c],
                                        op=alu.mult)
                w0b = sbw.tile([128, NBC], dt.bfloat16, tag="w0b", bufs=3)
                nc.vector.tensor_copy(w0b[:, :nb_c], w0[:, :nb_c])
                if cols > 0:
                    w = sbw.tile([128, CAPC], dt.float32, tag="w")
                    rd_b = rd[:, :nb_c].rearrange(
                        "p (b one) -> p b one", one=1).to_broadcast(
                        (128, nb_c, D))
                    nc.vector.tensor_tensor(
                        out=w[:, :cols].rearrange("p (b d) -> p b d", d=D),
                        in0=E[:, :cols].rearrange("p (b d) -> p b d", d=D),
                        in1=rd_b, op=alu.mult)
                    # pair-half weights: w1 = w*par ; w0h = w - w1
                    w1 = sbw.tile([128, CAPC], dt.float32, tag="w1")
                    nc.vector.tensor_tensor(out=w1[:, :cols], in0=w[:, :cols],
                                            in1=par_sb[:, csl], op=alu.mult)
                    nc.vector.tensor_tensor(out=w[:, :cols], in0=w[:, :cols],
                                            in1=w1[:, :cols], op=alu.subtract)
                    wLb = sbw.tile([128, CAPC], dt.bfloat16, tag="wLb", bufs=3)
                    wHb = sbw.tile([128, CAPC], dt.bfloat16, tag="wHb", bufs=3)
                    nc.vector.tensor_copy(wLb[:, :cols], w[:, :cols])
                    nc.vector.tensor_copy(wHb[:, :cols], w1[:, :cols])
                    for t in range(cols):
                        acc_mm(z[:, t * 2 * F:t * 2 * F + F],
                               wLb[:, t:t + 1])
                        acc_mm(z[:, t * 2 * F + F:(t + 1) * 2 * F],
                               wHb[:, t:t + 1])
                for bi in range(nb_c):
                    acc_mm(z0[:, bi * F:(bi + 1) * F], w0b[:, bi:bi + 1])

            # ---- tail:  S = pacc - xr_sum ; AllReduce ; head ----
            xrs_sb = sbp.tile([F, 1], dt.float32)
            nc.vector.tensor_copy(xrs_sb[:], xrs[:])
            part_sb = sbp.tile([F, 1], dt.float32)
            nc.vector.tensor_tensor(out=part_sb[:], in0=pacc[:], in1=xrs_sb[:],
                                    op=alu.subtract)
            nc.sync.dma_start(part_loc[:].rearrange("o f -> f o"), part_sb[:])

            nc.gpsimd.collective_compute(
                "AllReduce", alu.add, replica_groups=rg,
                ins=[part_loc.opt()], outs=[pooled.opt()])

            # ---- head ----
            pool_sb = sbp.tile([F, 1], dt.float32)
            nc.sync.dma_start(pool_sb[:], pooled[:].rearrange("o f -> f o"))
            Ap_sb = sbp.tile([F, 1], dt.float32)
            nc.sync.dma_start(Ap_sb[:], Ap)
            Bp_sb = sbp.tile([F, 1], dt.float32)
            nc.sync.dma_start(Bp_sb[:], Bp)
            Wc_sb = sbp.tile([F, NCLS], dt.float32)
            nc.sync.dma_start(Wc_sb[:], Wcp)
            bc_sb = sbp.tile([1, NCLS], dt.float32)
            nc.sync.dma_start(bc_sb[:], bc)
            h_sb = sbp.tile([F, 1], dt.float32)
            nc.vector.scalar_tensor_tensor(
                out=h_sb[:], in0=pool_sb[:], scalar=Ap_sb[:, 0:1], in1=Bp_sb[:],
                op0=alu.mult, op1=alu.add)
            one1 = sbp.tile([1, 1], dt.float32)
            nc.vector.memset(one1[:], 1.0)
            hp = pp1.tile([1, NCLS], dt.float32, tag="hp")
            nc.tensor.matmul(hp[:], lhsT=h_sb[:], rhs=Wc_sb[:], start=True,
                             stop=False)
            nc.tensor.matmul(hp[:], lhsT=one1[:], rhs=bc_sb[:], start=False,
                             stop=True)
            eh = sbp.tile([1, NCLS], dt.float32)
            nc.scalar.activation(eh[:], hp[:], act.Exp)
            den = sbp.tile([1, 1], dt.float32)
            nc.vector.tensor_reduce(den[:], eh[:], axis=mybir.AxisListType.X,
                                    op=alu.add)
            rden = sbp.tile([1, 1], dt.float32)
            nc.vector.reciprocal(rden[:], den[:])
            osb = sbp.tile([1, NCLS], dt.float32)
            nc.vector.tensor_scalar(out=osb[:], in0=eh[:], scalar1=rden[:, 0:1],
                                    scalar2=None, op0=alu.mult)
            nc.sync.dma_start(out, osb[:])

    nc.compile()
    return nc


# --------------------------------------------------------------------------
# public entry point
# --------------------------------------------------------------------------

_CACHE = {}


def _install_ntff_hook():
    """Provide antenv.axon_hooks + the ctypes NTFF hook when the image lacks
    them, so run_bass_kernel_spmd(trace=True) can capture exec_time_ns."""
    import contextlib
    import ctypes
    import sys
    import types

    try:
        import antenv.axon_hooks  # noqa: F401
        return
    except ImportError:
        pass
    try:
        import antenv
    except ImportError:
        return
    holder = [None]
    mod = types.ModuleType("antenv.axon_hooks")
    mod.set_axon_ntff_profile_hook = lambda h: holder.__setitem__(0, h)
    mod.get_axon_ntff_profile_hook = lambda: holder[0]
    sys.modules["antenv.axon_hooks"] = mod
    antenv.axon_hooks = mod

    so_path = "/opt/axon/libaxon_pjrt.so"
    if os.path.exists(so_path):
        lib = ctypes.CDLL(so_path)
        if hasattr(lib, "axon_start_nrt_profile"):
            lib.axon_start_nrt_profile.argtypes = [
                ctypes.POINTER(ctypes.c_int64), ctypes.c_size_t]
            lib.axon_start_nrt_profile.restype = ctypes.c_int64
            lib.axon_stop_nrt_profile.argtypes = [ctypes.c_char_p]
            lib.axon_stop_nrt_profile.restype = ctypes.c_int64

            @contextlib.contextmanager
            def _hook(output_dir, device_ids):
                import jax
                jax.devices()
                if device_ids:
                    ids = (ctypes.c_int64 * len(device_ids))(*device_ids)
                    rc = lib.axon_start_nrt_profile(ids, len(device_ids))
                else:
                    rc = lib.axon_start_nrt_profile(None, 0)
                if rc != 0:
                    raise RuntimeError(f"axon_start_nrt_profile rc={rc}")
                try:
                    yield
                finally:
                    n = lib.axon_stop_nrt_profile(str(output_dir).encode())
                    print(f"ntff profile: {n} file(s) -> {output_dir}")

            mod.set_axon_ntff_profile_hook(_hook)

    import concourse.bass_utils as bu
    bu.upload_artifacts = lambda tmpdir: "local://" + str(tmpdir)


def kernel(**inputs):
    from concourse.bass_utils import run_bass_kernel_spmd

    if bool(int(os.environ.get("KERNEL_TRACE", "0"))):
        _install_ntff_hook()
    inputs = {k: np.asarray(v) for k, v in inputs.items()}
    in_maps, meta = prep_host(**inputs)
    key = (meta["PP"], meta["TC"], meta["chunks"])
    if key not in _CACHE:
        _CACHE[key] = build(meta)
    nc = _CACHE[key]
    res = run_bass_kernel_spmd(nc, in_maps, core_ids=list(range(M)),
                               trace=bool(int(os.environ.get("KERNEL_TRACE", "0"))))
    if getattr(res, "exec_time_ns", None) is not None:
        print(f"HW exec time: {res.exec_time_ns} ns")
    return np.asarray(res.results[0]["out"]).astype(np.float32)
